# revision 23
# baseline (speedup 1.0000x reference)
"""Sparse-attention Trainium2 kernel (nn_Attention_81398220193933).

Strategy (tensor-parallel over heads, 2 heads per NeuronCore):
  - Host pre-lays-out per-core tensors:
      qT  [B, 128, S]  bf16 : rows 0:64 = headA Q^T / sqrt(dh), rows 64:128 = headB
      kT  [B, 128, S]  bf16 : same for K^T (only the first kb*128 keys loaded)
      vE  [B, 128, KT, 130] bf16 : per k-tile t, partition p = key position
           t*128+p, cols [0]=emb, [1:65]=V_A*emb, [65]=emb, [66:130]=V_B*emb
           where emb[b,k] = exp(bias[k]) * (k < seq_len[b]) (all-valid if
           seq_len==0). Folding the additive key bias + mask multiplicatively
           into V makes the softmax mask/bias free on-device and lets fully
           masked k-tiles be skipped.
  - Device, per batch b and key-tile t (kb = ceil(seq_len/128) tiles):
      scores^T [k=128, q=1024] f32 = K_tile^T.T @ Q^T for both heads; the two
          64-contraction-row matmuls run concurrently on separate PE-array
          row halves (explicit tile_position (0,0)/(64,0)).
      W^T = exp(scores^T): alternates between ScalarE (exact LUT exp -> bf16)
          and VectorE (one-instruction Schraudolph fast-exp: int16(x*128*log2e
          + magic) bitcast to bf16, ~+-2% weight error) to halve the exp wall.
      accT[66, q] += [emb|V]^T.T @ W^T: V is the *stationary* operand (65-col
          weight loads instead of 8x128) and W streams 2x512 columns; row 0
          accumulates the softmax denominator. Accumulated over t in PSUM,
          inline one step behind the QK stream (no drip queue needed).
    Epilogue per (batch, head): one PSUM->SBUF f32 copy (on whichever of
    ScalarE/VectorE is less loaded) + DMA of the transposed numerators and
    denominators; the host does the final divide + transpose (0.2% of FLOPs).
  - HAM keep-warm: junk matmuls write rows 96:128 of the live accumulator
    banks (start=False so the bank's has_written state is untouched), making
    them dependency-free; plus a startup burst during the initial DMA wait.
  - Softmax max-subtraction is unnecessary: logits are O(+-6) and masked keys
    contribute exactly zero through emb; a fully-masked row degenerates to
    softmax over all keys exactly like the jax reference.
  - PSUM: 2 score slots (2 banks each) + 2 head accumulators (2 banks each).
"""

import numpy as np
import ml_dtypes

import concourse.bass as bass
import concourse.mybir as mybir
import concourse.tile as tile
from concourse import bacc
from concourse.bass_utils import run_bass_kernel_spmd

B = 8
S = 1024
UNITS = 1024
H = 16
DH = 64
N_CORES = 8
KT = S // 128  # max key tiles per batch

BF16 = mybir.dt.bfloat16
F32 = mybir.dt.float32
I16 = mybir.dt.int16

# fast-exp: i16 = trunc/round(x * 128*log2e + (16256 - C)); bits viewed as bf16
EXP_SCALE = 128.0 * 1.4426950408889634
EXP_OFF = 16256.0 - 5.25
# engine cost model (us) used only for load balancing between ScalarE/VectorE
ACT_EXP, DVE_EXP = 1.15, 2.17  # VectorE pays a pipeline DRAIN between ops
ACT_CPY, DVE_CPY = 1.0, 2.15


def _build_nc(kbs):
    """Build the SPMD Bass program. kbs: per-batch number of 128-key tiles."""
    nc = bacc.Bacc("TRN2", target_bir_lowering=False, debug=False,
                   num_devices=N_CORES)
    qT = nc.dram_tensor("qt", [B, 128, S], BF16, kind="ExternalInput").ap()
    kT = nc.dram_tensor("kt", [B, 128, S], BF16, kind="ExternalInput").ap()
    vE = nc.dram_tensor("vt", [B, 128, KT, 130], BF16, kind="ExternalInput").ap()
    o = nc.dram_tensor("o", [B, 2, 65, S], F32, kind="ExternalOutput").ap()

    with tile.TileContext(nc) as tc:
        with (
            tc.tile_pool(name="qk", bufs=2) as qk_pool,
            tc.tile_pool(name="v", bufs=2) as v_pool,
            tc.tile_pool(name="wa", bufs=16) as wa_pool,
            tc.tile_pool(name="wd", bufs=10) as wd_pool,
            tc.tile_pool(name="ot", bufs=4) as o_pool,
            tc.tile_pool(name="sc", bufs=3, space="PSUM") as sc_pool,
            tc.tile_pool(name="acc", bufs=1, space="PSUM") as acc_pool,
        ):
            bal = {"act": 0.0, "dve": 0.0}

            def emit_exp(sc):
                """exp of one [128, S] score tile on the less-loaded engine."""
                if bal["act"] <= bal["dve"]:
                    bal["act"] += ACT_EXP
                    wt = wa_pool.tile([128, S], BF16, tag="wa", name="wa")
                    nc.scalar.activation(wt[:], sc[:],
                                         mybir.ActivationFunctionType.Exp)
                    return wt
                bal["dve"] += DVE_EXP
                wt = wd_pool.tile([128, S], I16, tag="wd", name="wd")
                nc.vector.tensor_scalar(
                    wt[:], sc[:], EXP_SCALE, EXP_OFF,
                    mybir.AluOpType.mult, mybir.AluOpType.add)
                return wt

            def w_ap(wt):
                ap = wt[:]
                return ap.bitcast(BF16) if wt.tensor.dtype == I16 else ap

            # Lazy A-V burst drip: heads enqueue at batch START; a drip
            # unit emits one key-tile's accumulation as soon as its W tile
            # has been exp'd, through the single 2-bank accumulator. This
            # keeps the PE dense from the very first steps (HAM warm) while
            # the 3 score slots decouple QK from the exp chain.
            burstq = []
            drip_state = {"cur": None}

            def drip(n):
                for _ in range(n):
                    cur = drip_state["cur"]
                    if cur is None:
                        if not burstq:
                            return
                        cand = burstq[0]
                        if not cand["rec"]["wts"][cand["h"]]:
                            return  # first W of this head not produced yet
                        cur = drip_state["cur"] = burstq.pop(0)
                        cur["acc"] = acc_pool.tile(
                            [128, S], F32, tag="acc",
                            name=f"acc{cur['rec']['b']}_{cur['h']}")
                    rec, h, t = cur["rec"], cur["h"], cur["t"]
                    if t >= len(rec["wts"][h]):
                        return  # W for this tile not produced yet
                    acc, kb = cur["acc"], rec["kb"]
                    wap = w_ap(rec["wts"][h][t])
                    for qc in range(2):
                        nc.tensor.matmul(
                            acc[0:65, qc * 512:(qc + 1) * 512],
                            lhsT=rec["vt"][:, t, h * 65:h * 65 + 65],
                            rhs=wap[:, qc * 512:(qc + 1) * 512],
                            start=(t == 0), stop=(t == kb - 1),
                        )
                    cur["t"] += 1
                    if cur["t"] == kb:
                        epilogue(rec, h, acc)
                        drip_state["cur"] = None

            def epilogue(rec, h, acc):
                """Numerators+denominators PSUM -> SBUF, one half per engine
                concurrently (small ops slot between exps) -> HBM."""
                ot = o_pool.tile([65, S], F32, tag="ot", name="ot")
                nc.scalar.copy(ot[:, 0:512], acc[0:65, 0:512])
                nc.vector.tensor_copy(ot[:, 512:1024], acc[0:65, 512:1024])
                bal["act"] += 0.60
                bal["dve"] += 1.10
                nc.sync.dma_start(out=o[rec["b"], h], in_=ot[:])

            # Preload the exp table-set (~2.7us) while the first DMAs fly.
            wexp = qk_pool.tile([1, 8], F32, tag="wexp", name="wexp", bufs=1)
            nc.vector.memset(wexp[:], 0.0)
            nc.scalar.activation(wexp[:], wexp[:],
                                 mybir.ActivationFunctionType.Exp)
            wu = qk_pool.tile([128, 640], BF16, tag="wu", name="wu", bufs=1)
            nc.vector.memset(wu[:], 0.0)

            def keep_warm(out_ap, lhsT, n, start=False):
                for _ in range(n):
                    nc.tensor.matmul(out_ap, lhsT=lhsT, rhs=wu[:, 128:640],
                                     start=start, stop=start,
                                     skip_group_check=True)

            # Load every batch's inputs up front (fits easily in SBUF) so no
            # QK phase ever waits on DMA. First batch small (warms up on real
            # work at low cost), then largest-first, smallest last (short tail
            # after the final exp).
            srt = sorted(range(B), key=lambda i: -kbs[i])
            order = [srt[-2]] + srt[:-2] + [srt[-1]]
            qts, kts, vts = {}, {}, {}
            for b in order:
                qts[b] = qk_pool.tile([128, S], BF16, tag=f"qt{b}",
                                      name=f"qt{b}", bufs=1)
                nc.sync.dma_start(out=qts[b][:], in_=qT[b])
                kts[b] = qk_pool.tile([128, kbs[b] * 128], BF16, tag=f"kt{b}",
                                      name=f"kt{b}", bufs=1)
                nc.sync.dma_start(out=kts[b][:], in_=kT[b, :, :kbs[b] * 128])
            for b in order:
                vts[b] = v_pool.tile([128, kbs[b], 130], BF16, tag=f"vt{b}",
                                     name=f"vt{b}", bufs=1)
                nc.sync.dma_start(out=vts[b][:], in_=vE[b, :, :kbs[b], :])

            # Startup burst: ~3.3us of junk matmuls during the initial DMA
            # wait fills one full HAM activity window, so the PE is at 2.4GHz
            # from the first real matmul on.
            scw = sc_pool.tile([128, S], F32, tag="sc", name="scwarm")
            for i in range(8):
                nc.tensor.matmul(scw[:, 256 * (i % 4):256 * (i % 4) + 256],
                                 lhsT=wu[:, 0:128], rhs=wu[:, 128:384],
                                 start=True, stop=True,
                                 skip_group_check=True)

            step_no = 0
            for bi, b in enumerate(order):
                kb = kbs[b]
                qt, kt, vt = qts[b], kts[b], vts[b]
                rec = {"b": b, "kb": kb, "wts": [[], []], "vt": vt}
                burstq.append({"rec": rec, "h": 0, "t": 0, "acc": None})
                burstq.append({"rec": rec, "h": 1, "t": 0, "acc": None})
                for t in range(kb):
                    # both heads' QK back-to-back at tile_position (0,0) /
                    # (64,0): concurrent on separate PE-array row halves.
                    scs = [sc_pool.tile([128, S], F32, tag="sc", name="sc")
                           for _ in range(2)]
                    # HAM keep-warm pulse: a small junk matmul into the score
                    # slot just before its real QK overwrites it (~107ns).
                    if step_no > 0:
                        npulse = 3 if step_no < 10 else 2
                        for j in range(npulse):
                            nc.tensor.matmul(
                                scs[0][:, 256 * (j % 2):256 * (j % 2) + 256],
                                lhsT=wu[:, 0:128], rhs=wu[:, 128:384],
                                start=True, stop=True,
                                skip_group_check=True)
                    step_no += 1
                    for qc in range(2):
                        for h in range(2):
                            base = 64 * h
                            nc.tensor.matmul(
                                scs[h][:, qc * 512:(qc + 1) * 512],
                                lhsT=kt[base:base + 64, t * 128:(t + 1) * 128],
                                rhs=qt[base:base + 64, qc * 512:(qc + 1) * 512],
                                start=True, stop=True,
                                tile_position=(base, 0),
                            )
                    ea = bal["act"] <= bal["dve"]
                    if ea:
                        rec["wts"][1].append(emit_exp(scs[1]))
                        rec["wts"][0].append(emit_exp(scs[0]))
                    else:
                        rec["wts"][0].append(emit_exp(scs[0]))
                        rec["wts"][1].append(emit_exp(scs[1]))
                    drip(3 if len(burstq) > 2 else 2)

            while burstq or drip_state["cur"] is not None:
                drip(1)
    nc.compile()
    return nc


_NC_CACHE = {}


def _get_nc(kbs):
    key = tuple(kbs)
    if key not in _NC_CACHE:
        _NC_CACHE[key] = _build_nc(key)
    return _NC_CACHE[key]


def kernel(memory, query, b, seq_len):
    memory = np.asarray(memory)
    query = np.asarray(query)
    bias = np.asarray(b, dtype=np.float32)
    seq_len = np.asarray(seq_len).reshape(-1).astype(np.int64)

    sl = seq_len.copy()
    kbs = [int(min(KT, max(1, -(-int(s) // 128)))) if s > 0 else KT for s in sl]

    # emb[b, k] = exp(bias[k]) * valid; fully-masked batch -> plain softmax
    pos = np.arange(S)[None, :]
    valid = (pos < sl[:, None]) | (sl[:, None] == 0)
    emb = np.exp(bias)[None, :] * valid.astype(np.float32)  # [B, S]

    qh = (query.astype(np.float32) * (DH ** -0.5)).reshape(B, S, H, DH)
    kh = memory[:, :, :UNITS].astype(np.float32).reshape(B, S, H, DH)
    vh = memory[:, :, UNITS:].astype(np.float32).reshape(B, S, H, DH)
    vh = vh * emb[:, :, None, None]  # [B, S, H, DH] value rows pre-masked

    bf = ml_dtypes.bfloat16
    # [B, S, H, DH] -> [B, H, DH, S] transposed layouts
    qTfull = np.ascontiguousarray(qh.transpose(0, 2, 3, 1)).astype(bf)
    kTfull = np.ascontiguousarray(kh.transpose(0, 2, 3, 1)).astype(bf)
    # [B, S, H, DH] -> [B, (t p), H, DH] -> [B, 128, KT, H, DH]
    vtiles = np.ascontiguousarray(
        vh.reshape(B, KT, 128, H, DH).transpose(0, 2, 1, 3, 4)).astype(bf)
    embt = np.ascontiguousarray(
        emb.reshape(B, KT, 128).transpose(0, 2, 1)).astype(bf)  # [B, 128, KT]

    in_maps = []
    for c in range(N_CORES):
        hA, hB = 2 * c, 2 * c + 1
        qTc = np.concatenate([qTfull[:, hA], qTfull[:, hB]], axis=1)
        kTc = np.concatenate([kTfull[:, hA], kTfull[:, hB]], axis=1)
        vEc = np.empty((B, 128, KT, 130), dtype=bf)
        vEc[..., 0] = embt
        vEc[..., 1:65] = vtiles[:, :, :, hA, :]
        vEc[..., 65] = embt
        vEc[..., 66:130] = vtiles[:, :, :, hB, :]
        in_maps.append({
            "qt": np.ascontiguousarray(qTc),
            "kt": np.ascontiguousarray(kTc),
            "vt": np.ascontiguousarray(vEc),
        })

    nc = _get_nc(kbs)
    res = run_bass_kernel_spmd(nc, in_maps, core_ids=list(range(N_CORES)))

    out = np.empty((B, S, UNITS), dtype=np.float32)
    for c in range(N_CORES):
        # o [B, 2, 65, S]: row 0 = denominators, rows 1:65 = numerators^T
        oc = np.asarray(res.results[c]["o"], dtype=np.float32)
        for h in range(2):
            num = oc[:, h, 1:65, :]            # [B, 64, S]
            den = oc[:, h, 0:1, :]             # [B, 1, S]
            out[:, :, 128 * c + 64 * h:128 * c + 64 * h + 64] = (
                num / den).transpose(0, 2, 1)
    return out


# revision 25
# speedup vs baseline: 1.2677x; 1.2677x over previous
"""Sparse-attention Trainium2 kernel (nn_Attention_81398220193933).

Strategy (tensor-parallel over heads, 2 heads per NeuronCore):
  - Host pre-lays-out per-core tensors:
      qT  [B, 128, S]  bf16 : rows 0:64 = headA Q^T / sqrt(dh), rows 64:128 = headB
      kT  [B, 128, S]  bf16 : same for K^T (only the first kb*128 keys loaded)
      vE  [B, 128, KT, 130] bf16 : per k-tile t, partition p = key position
           t*128+p, cols [0]=emb, [1:65]=V_A*emb, [65]=emb, [66:130]=V_B*emb
           where emb[b,k] = exp(bias[k]) * (k < seq_len[b]) (all-valid if
           seq_len==0). Folding the additive key bias + mask multiplicatively
           into V makes the softmax mask/bias free on-device and lets fully
           masked k-tiles be skipped.
  - Device, per batch b and key-tile t (kb = ceil(seq_len/128) tiles):
      scores^T [k=128, q=1024] f32 = K_tile^T.T @ Q^T for both heads; the two
          64-contraction-row matmuls run concurrently on separate PE-array
          row halves (explicit tile_position (0,0)/(64,0)).
      W^T = exp(scores^T): alternates between ScalarE (exact LUT exp -> bf16)
          and VectorE (one-instruction Schraudolph fast-exp: int16(x*128*log2e
          + magic) bitcast to bf16, ~+-2% weight error) to halve the exp wall.
      accT[66, q] += [emb|V]^T.T @ W^T: V is the *stationary* operand (65-col
          weight loads instead of 8x128) and W streams 2x512 columns; row 0
          accumulates the softmax denominator. Accumulated over t in PSUM,
          inline one step behind the QK stream (no drip queue needed).
    Epilogue per (batch, head): one PSUM->SBUF f32 copy (on whichever of
    ScalarE/VectorE is less loaded) + DMA of the transposed numerators and
    denominators; the host does the final divide + transpose (0.2% of FLOPs).
  - HAM keep-warm: junk matmuls write rows 96:128 of the live accumulator
    banks (start=False so the bank's has_written state is untouched), making
    them dependency-free; plus a startup burst during the initial DMA wait.
  - Softmax max-subtraction is unnecessary: logits are O(+-6) and masked keys
    contribute exactly zero through emb; a fully-masked row degenerates to
    softmax over all keys exactly like the jax reference.
  - PSUM: 2 score slots (2 banks each) + 2 head accumulators (2 banks each).
"""

import numpy as np
import ml_dtypes

import concourse.bass as bass
import concourse.mybir as mybir
import concourse.tile as tile
from concourse import bacc
from concourse.bass_utils import run_bass_kernel_spmd

B = 8
S = 1024
UNITS = 1024
H = 16
DH = 64
N_CORES = 8
KT = S // 128  # max key tiles per batch

BF16 = mybir.dt.bfloat16
F32 = mybir.dt.float32
I16 = mybir.dt.int16

# fast-exp: i16 = trunc/round(x * 128*log2e + (16256 - C)); bits viewed as bf16
EXP_SCALE = 128.0 * 1.4426950408889634
EXP_OFF = 16256.0 - 5.25
# engine cost model (us) used only for load balancing between ScalarE/VectorE
ACT_EXP, DVE_EXP = 1.147, 1.192
ACT_CPY, DVE_CPY = 1.0, 1.19


def _build_nc(kbs):
    """Build the SPMD Bass program. kbs: per-batch number of 128-key tiles."""
    nc = bacc.Bacc("TRN2", target_bir_lowering=False, debug=False,
                   num_devices=N_CORES)
    qT = nc.dram_tensor("qt", [B, 128, S], BF16, kind="ExternalInput").ap()
    kT = nc.dram_tensor("kt", [B, 128, S], BF16, kind="ExternalInput").ap()
    vE = nc.dram_tensor("vt", [B, 128, KT, 130], BF16, kind="ExternalInput").ap()
    o = nc.dram_tensor("o", [B, 2, 65, S], F32, kind="ExternalOutput").ap()

    with tile.TileContext(nc) as tc:
        with (
            tc.tile_pool(name="qk", bufs=2) as qk_pool,
            tc.tile_pool(name="v", bufs=2) as v_pool,
            tc.tile_pool(name="wa", bufs=4) as wa_pool,
            tc.tile_pool(name="wd", bufs=4) as wd_pool,
            tc.tile_pool(name="ot", bufs=4) as o_pool,
            tc.tile_pool(name="sc", bufs=2, space="PSUM") as sc_pool,
            tc.tile_pool(name="acc", bufs=2, space="PSUM") as acc_pool,
        ):
            bal = {"act": 0.0, "dve": 0.0}

            def emit_exp(sc):
                """exp of one [128, S] score tile on the less-loaded engine."""
                if bal["act"] <= bal["dve"]:
                    bal["act"] += ACT_EXP
                    wt = wa_pool.tile([128, S], BF16, tag="wa", name="wa")
                    nc.scalar.activation(wt[:], sc[:],
                                         mybir.ActivationFunctionType.Exp)
                    return wt
                bal["dve"] += DVE_EXP
                wt = wd_pool.tile([128, S], I16, tag="wd", name="wd")
                nc.vector.tensor_scalar(
                    wt[:], sc[:], EXP_SCALE, EXP_OFF,
                    mybir.AluOpType.mult, mybir.AluOpType.add)
                return wt

            def w_ap(wt):
                ap = wt[:]
                return ap.bitcast(BF16) if wt.tensor.dtype == I16 else ap

            def emit_av(p, t, stop):
                """A-V accumulation for key-tile t of both heads: V stationary
                (65-col weight load), W^T streaming 2x512 columns."""
                for h in range(2):
                    acc = p["acc"][h]
                    wap = w_ap(p["wts"][h][t])
                    for qc in range(2):
                        nc.tensor.matmul(
                            acc[0:65, qc * 512:(qc + 1) * 512],
                            lhsT=p["vt"][:, t, h * 65:h * 65 + 65],
                            rhs=wap[:, qc * 512:(qc + 1) * 512],
                            start=(t == 0), stop=stop,
                        )

            def epilogue(p, h):
                """Copy numerators+denominators PSUM -> SBUF -> HBM."""
                acc = p["acc"][h]
                ot = o_pool.tile([65, S], F32, tag="ot", name="ot")
                if bal["act"] <= bal["dve"]:
                    bal["act"] += ACT_CPY
                    nc.scalar.copy(ot[:], acc[0:65, :])
                else:
                    bal["dve"] += DVE_CPY
                    nc.vector.tensor_copy(ot[:], acc[0:65, :])
                nc.sync.dma_start(out=o[p["b"], h], in_=ot[:])

            # Preload the exp table-set (~2.7us) while the first DMAs fly.
            wexp = qk_pool.tile([1, 8], F32, tag="wexp", name="wexp", bufs=1)
            nc.vector.memset(wexp[:], 0.0)
            nc.scalar.activation(wexp[:], wexp[:],
                                 mybir.ActivationFunctionType.Exp)
            wu = qk_pool.tile([128, 640], BF16, tag="wu", name="wu", bufs=1)
            nc.vector.memset(wu[:], 0.0)

            def keep_warm(out_ap, lhsT, n, start=False):
                for _ in range(n):
                    nc.tensor.matmul(out_ap, lhsT=lhsT, rhs=wu[:, 128:640],
                                     start=start, stop=start,
                                     skip_group_check=True)

            # Load every batch's inputs up front (fits easily in SBUF) so no
            # QK phase ever waits on DMA. First batch small (warms up on real
            # work at low cost), then largest-first, smallest last (short tail
            # after the final exp).
            srt = sorted(range(B), key=lambda i: -kbs[i])
            order = [srt[-2]] + srt[:-2] + [srt[-1]]
            qts, kts, vts = {}, {}, {}
            for b in order:
                qts[b] = qk_pool.tile([128, S], BF16, tag=f"qt{b}",
                                      name=f"qt{b}", bufs=1)
                nc.sync.dma_start(out=qts[b][:], in_=qT[b])
                kts[b] = qk_pool.tile([128, kbs[b] * 128], BF16, tag=f"kt{b}",
                                      name=f"kt{b}", bufs=1)
                nc.sync.dma_start(out=kts[b][:], in_=kT[b, :, :kbs[b] * 128])
            for b in order:
                vts[b] = v_pool.tile([128, kbs[b], 130], BF16, tag=f"vt{b}",
                                     name=f"vt{b}", bufs=1)
                nc.sync.dma_start(out=vts[b][:], in_=vE[b, :, :kbs[b], :])

            # Startup burst: ~3.3us of junk matmuls during the initial DMA
            # wait fills one full HAM activity window, so the PE is at 2.4GHz
            # from the first real matmul on.
            scw = sc_pool.tile([128, S], F32, tag="sc", name="scwarm")
            for i in range(8):
                nc.tensor.matmul(scw[:, 256 * (i % 4):256 * (i % 4) + 256],
                                 lhsT=wu[:, 0:128], rhs=wu[:, 128:384],
                                 start=True, stop=True,
                                 skip_group_check=True)

            prev = None  # batch still owing its last AV + epilogues
            step_no = 0
            for bi, b in enumerate(order):
                kb = kbs[b]
                qt, kt, vt = qts[b], kts[b], vts[b]
                rec = {"b": b, "kb": kb, "wts": [[], []], "vt": vt,
                       "acc": [None, None]}
                for t in range(kb):
                    # both heads' QK back-to-back at tile_position (0,0) /
                    # (64,0): concurrent on separate PE-array row halves.
                    scs = [sc_pool.tile([128, S], F32, tag="sc", name="sc")
                           for _ in range(2)]
                    # HAM keep-warm pulse: a small junk matmul into the score
                    # slot just before its real QK overwrites it (~107ns).
                    if step_no > 0:
                        npulse = 3 if step_no < 12 else 0
                        for j in range(npulse):
                            nc.tensor.matmul(
                                scs[0][:, 256 * (j % 2):256 * (j % 2) + 256],
                                lhsT=wu[:, 0:128], rhs=wu[:, 128:384],
                                start=True, stop=True,
                                skip_group_check=True)
                    step_no += 1
                    for qc in range(2):
                        for h in range(2):
                            base = 64 * h
                            nc.tensor.matmul(
                                scs[h][:, qc * 512:(qc + 1) * 512],
                                lhsT=kt[base:base + 64, t * 128:(t + 1) * 128],
                                rhs=qt[base:base + 64, qc * 512:(qc + 1) * 512],
                                start=True, stop=True,
                                tile_position=(base, 0),
                            )
                    # DVE fast-exp emitted first (it is the longer op and has
                    # its own pool; keeps the two engines decoupled)
                    ea = bal["act"] <= bal["dve"]
                    if ea:
                        rec["wts"][1].append(emit_exp(scs[1]))
                        rec["wts"][0].append(emit_exp(scs[0]))
                    else:
                        rec["wts"][0].append(emit_exp(scs[0]))
                        rec["wts"][1].append(emit_exp(scs[1]))
                    # finish the previous batch: its last AV + epilogues
                    if prev is not None:
                        emit_av(prev, prev["kb"] - 1, stop=True)
                        epilogue(prev, 0)
                        epilogue(prev, 1)
                        prev = None
                    if t == 0:
                        rec["acc"] = [
                            acc_pool.tile([128, S], F32, tag="acc",
                                          name=f"acc{b}_{h}")
                            for h in range(2)]
                    else:
                        emit_av(rec, t - 1, stop=False)
                prev = rec

            emit_av(prev, prev["kb"] - 1, stop=True)
            epilogue(prev, 0)
            epilogue(prev, 1)
    nc.compile()
    return nc


_NC_CACHE = {}


def _get_nc(kbs):
    key = tuple(kbs)
    if key not in _NC_CACHE:
        _NC_CACHE[key] = _build_nc(key)
    return _NC_CACHE[key]


def kernel(memory, query, b, seq_len):
    memory = np.asarray(memory)
    query = np.asarray(query)
    bias = np.asarray(b, dtype=np.float32)
    seq_len = np.asarray(seq_len).reshape(-1).astype(np.int64)

    sl = seq_len.copy()
    kbs = [int(min(KT, max(1, -(-int(s) // 128)))) if s > 0 else KT for s in sl]

    # emb[b, k] = exp(bias[k]) * valid; fully-masked batch -> plain softmax
    pos = np.arange(S)[None, :]
    valid = (pos < sl[:, None]) | (sl[:, None] == 0)
    emb = np.exp(bias)[None, :] * valid.astype(np.float32)  # [B, S]

    qh = (query.astype(np.float32) * (DH ** -0.5)).reshape(B, S, H, DH)
    kh = memory[:, :, :UNITS].astype(np.float32).reshape(B, S, H, DH)
    vh = memory[:, :, UNITS:].astype(np.float32).reshape(B, S, H, DH)
    vh = vh * emb[:, :, None, None]  # [B, S, H, DH] value rows pre-masked

    bf = ml_dtypes.bfloat16
    # [B, S, H, DH] -> [B, H, DH, S] transposed layouts
    qTfull = np.ascontiguousarray(qh.transpose(0, 2, 3, 1)).astype(bf)
    kTfull = np.ascontiguousarray(kh.transpose(0, 2, 3, 1)).astype(bf)
    # [B, S, H, DH] -> [B, (t p), H, DH] -> [B, 128, KT, H, DH]
    vtiles = np.ascontiguousarray(
        vh.reshape(B, KT, 128, H, DH).transpose(0, 2, 1, 3, 4)).astype(bf)
    embt = np.ascontiguousarray(
        emb.reshape(B, KT, 128).transpose(0, 2, 1)).astype(bf)  # [B, 128, KT]

    in_maps = []
    for c in range(N_CORES):
        hA, hB = 2 * c, 2 * c + 1
        qTc = np.concatenate([qTfull[:, hA], qTfull[:, hB]], axis=1)
        kTc = np.concatenate([kTfull[:, hA], kTfull[:, hB]], axis=1)
        vEc = np.empty((B, 128, KT, 130), dtype=bf)
        vEc[..., 0] = embt
        vEc[..., 1:65] = vtiles[:, :, :, hA, :]
        vEc[..., 65] = embt
        vEc[..., 66:130] = vtiles[:, :, :, hB, :]
        in_maps.append({
            "qt": np.ascontiguousarray(qTc),
            "kt": np.ascontiguousarray(kTc),
            "vt": np.ascontiguousarray(vEc),
        })

    nc = _get_nc(kbs)
    res = run_bass_kernel_spmd(nc, in_maps, core_ids=list(range(N_CORES)))

    out = np.empty((B, S, UNITS), dtype=np.float32)
    for c in range(N_CORES):
        # o [B, 2, 65, S]: row 0 = denominators, rows 1:65 = numerators^T
        oc = np.asarray(res.results[c]["o"], dtype=np.float32)
        for h in range(2):
            num = oc[:, h, 1:65, :]            # [B, 64, S]
            den = oc[:, h, 0:1, :]             # [B, 1, S]
            out[:, :, 128 * c + 64 * h:128 * c + 64 * h + 64] = (
                num / den).transpose(0, 2, 1)
    return out
